# revision 21
# baseline (speedup 1.0000x reference)
"""PatchMatch-style MatchingPropagator on 8 Trainium2 NeuronCores.

Full inputs in, full outputs out. Sharding: 8 independent units =
(direction in {forward, backward}) x (batch 0..3), one NeuronCore each.
Core b runs forward for batch b; core 4+b runs backward for batch b using
the host-transposed correlation volume.

v2: the correlation volume is expanded on the host into a "quad" layout
corr_q[pixel, y0, x0] = [v00, v01, v10, v11] so one bilinear sample is a
single 16-byte indirect-DMA descriptor (half the SWDGE descriptor-
generation work, which is the dominant serial cost). The initial eval and
the first propagate depend only on the inputs, so their gather indices and
weight quads are precomputed on the host and shipped with the state; both
gathers issue as soon as the small index DMA lands. Scores are computed in
5 wide DVE ops via broadcast views of interleaved weight quads
[u, wx, t, wy], keeping the reference's exact fp32 multiply/add tree so
every argmax decision matches the reference bitwise. The gather address
chain runs on the Pool engine (which also issues the gather).

Pixel layout on chip: pixel (i, j) -> partition 64*(j//32) + i, free j%32.
"""

import numpy as np

B, H, W = 4, 64, 64
R = 3.0
EPS = np.float32(0.01)
N_CORES = 8
PIX = H * W  # 4096 pixels per unit; each owns a 64x64 correlation map
M_RNE = float(1 << 23)

_CACHE = {}


# ----------------------------------------------------------------------------
# Device program (SPMD: identical on all 8 cores; data differs per core)
# ----------------------------------------------------------------------------

def _build_program():
    import concourse.bass as bass
    import concourse.mybir as mybir
    import concourse.tile as tile
    from concourse import bacc

    F32 = mybir.dt.float32
    BF16 = mybir.dt.bfloat16
    I32 = mybir.dt.int32
    OP = mybir.AluOpType
    AF = mybir.ActivationFunctionType

    nc = bacc.Bacc(
        "TRN2",
        target_bir_lowering=False,
        debug=False,
        enable_asserts=False,
        num_devices=N_CORES,
    )

    # quad corr: row r = pixel*4096 + y0*64 + x0 -> [v00, v01, v10, v11]
    corr = nc.dram_tensor("corr", [PIX * PIX, 4], F32, kind="ExternalInput")
    # state rows: 0 x, 1 y, 2 base, 3..8 noise (nx,ny)*3, 9..12 W40,
    # 13..20 W41, 21..22 cand_h xy, 23..24 cand_v xy
    state_in = nc.dram_tensor("state", [25, 128, 32], F32,
                              kind="ExternalInput")
    idx_in = nc.dram_tensor("idx", [128, 96], I32, kind="ExternalInput")
    # partition-shift permutation matrices for the v-roll (dy=+1, dy=-1);
    # 0.0/1.0 are exact in bf16, so a bf16 stationary keeps the matmul exact
    perm_in = nc.dram_tensor("perm", [2, 128, 128], F32,
                             kind="ExternalInput")
    out_xy = nc.dram_tensor("out_xy", [2, 128, 32], F32,
                            kind="ExternalOutput")

    corr_ap = corr.ap()

    def b3(ap):  # [128,32] -> broadcast [128,3,32]
        return ap.rearrange("p (one f) -> p one f", one=1).to_broadcast(
            [128, 3, 32])

    with tile.TileContext(nc) as tc:
        with tc.tile_pool(name="main", bufs=1) as pool, \
                tc.tile_pool(name="ps", bufs=1,
                             space=bass.MemorySpace.PSUM) as ppool:
            IDX = pool.tile([128, 96], I32, name="IDX")
            state = pool.tile([128, 25 * 32], F32, name="state")
            PERM = pool.tile([128, 256], F32, name="PERM")
            PS = ppool.tile([128, 64], F32, name="PS")
            nc.sync.dma_start(IDX[:], idx_in.ap())
            nc.sync.dma_start(
                state[:].rearrange("p (n f) -> p n f", n=25),
                state_in.ap().rearrange("n p f -> p n f"),
            )
            nc.sync.dma_start(
                PERM[:].rearrange("p (n f) -> p n f", n=2),
                perm_in.ap().rearrange("n p f -> p n f"),
            )
            base_row = state[:, 64:96]

            def noise_view(k):
                o = 96 + 64 * k
                return state[:, o:o + 64]  # [nx|ny]

            BEST = pool.tile([128, 96], F32, name="BEST")    # [x|y|s]
            CAND = pool.tile([128, 192], F32, name="CAND")   # [hx hy hs|vx vy vs]
            RC = pool.tile([128, 96], F32, name="RC")        # [x y s]
            WF = pool.tile([128, 128], F32, name="WF")
            X0 = pool.tile([128, 128], F32, name="X0")
            W4 = pool.tile([128, 256], F32, name="W4")       # [u wx t wy]*
            IF = pool.tile([128, 64], F32, name="IF")
            I = pool.tile([128, 64], I32, name="I")
            G = pool.tile([128, 384], F32, name="G")         # single|dual
            At = pool.tile([128, 256], F32, name="At")
            Bt = pool.tile([128, 256], F32, name="Bt")
            D1 = pool.tile([128, 64], F32, name="D1")
            D2 = pool.tile([128, 64], F32, name="D2")
            UPD = pool.tile([128, 96], I32, name="UPD")

            v = nc.vector

            def emit_gather(icols_off, n_idx, gcols_off):
                nc.gpsimd.indirect_dma_start(
                    out=G[:, gcols_off:gcols_off + 4 * n_idx],
                    out_offset=None,
                    in_=corr_ap,
                    in_offset=bass.IndirectOffsetOnAxis(
                        ap=IDX[:, icols_off:icols_off + n_idx], axis=0),
                )

            def emit_gather_I(n_idx, gcols_off):
                nc.gpsimd.indirect_dma_start(
                    out=G[:, gcols_off:gcols_off + 4 * n_idx],
                    out_offset=None,
                    in_=corr_ap,
                    in_offset=bass.IndirectOffsetOnAxis(
                        ap=I[:, 0:n_idx], axis=0),
                )

            def one1(ap):  # append a trailing size-1 dim
                return ap.rearrange("... (f one) -> ... f one", one=1)

            def emit_score(g_flat, w_flat, out_v, a, eng=None, scr=0):
                """bilinear scores for `a` candidates: 5 ops, exact fp32
                tree ((v00*u)*t + (v01*wx)*t + (v10*u)*wy) + (v11*wx)*wy
                evaluated left-to-right like the reference. `scr` picks a
                scratch half so two engines can score concurrently."""
                e = eng or v
                g = g_flat.rearrange("p (a i r c) -> p a i r c",
                                     a=a, i=32, r=2, c=2)
                w4v = w_flat.rearrange("p (a i k m) -> p a i k m",
                                       a=a, i=32, k=2, m=2)
                uw = w4v[:, :, :, 0:1, :].to_broadcast([128, a, 32, 2, 2])
                twy = (w4v[:, :, :, 1:2, :]
                       .rearrange("p a i k m -> p a i m k")
                       .to_broadcast([128, a, 32, 2, 2]))
                ao, do = 128 * scr, 32 * scr
                atv = At[:, ao:ao + a * 128].rearrange(
                    "p (a i r c) -> p a i r c", a=a, i=32, r=2, c=2)
                btv = Bt[:, ao:ao + a * 128].rearrange(
                    "p (a i r c) -> p a i r c", a=a, i=32, r=2, c=2)
                e.tensor_tensor(atv, g, uw, OP.mult)
                e.tensor_tensor(btv, atv, twy, OP.mult)
                q = Bt[:, ao:ao + a * 128].rearrange(
                    "p (a i q) -> p a i q", a=a, i=32, q=4)
                d1 = one1(D1[:, do:do + a * 32]
                          .rearrange("p (a f) -> p a f", a=a))
                d2 = one1(D2[:, do:do + a * 32]
                          .rearrange("p (a f) -> p a f", a=a))
                e.tensor_tensor(d1, q[:, :, :, 0:1], q[:, :, :, 1:2], OP.add)
                e.tensor_tensor(d2, d1, q[:, :, :, 2:3], OP.add)
                e.tensor_tensor(one1(out_v), d2, q[:, :, :, 3:4], OP.add)

            def emit_floor(cv, a, off):
                """cv: coords view [128, a, 2(k=x/y), 32]. Clamped floor ->
                X0[:, off:off+a*64]."""
                wfv = WF[:, off:off + a * 64].rearrange(
                    "p (a k f) -> p a k f", a=a, k=2)
                x0v = X0[:, off:off + a * 64].rearrange(
                    "p (a k f) -> p a k f", a=a, k=2)
                v.tensor_scalar(wfv, cv, M_RNE, M_RNE, OP.add, OP.subtract)
                v.tensor_tensor(x0v, wfv, cv, OP.is_gt)
                v.tensor_tensor(x0v, wfv, x0v, OP.subtract)
                v.tensor_scalar(x0v, x0v, float(W - 2), None, OP.min)

            def emit_addr(a):
                """int32 quad indices -> I from X0[:, 0:a*64]."""
                x0v = X0[:, 0:a * 64].rearrange("p (a k f) -> p a k f",
                                                a=a, k=2)
                ifv = IF[:, 0:a * 32].rearrange("p (a one f) -> p a one f",
                                                a=a, one=1)
                iv = I[:, 0:a * 32].rearrange("p (a one f) -> p a one f",
                                              a=a, one=1)
                basev = (base_row.rearrange("p (one onee f) -> p one onee f",
                                            one=1, onee=1)
                         .to_broadcast([128, a, 1, 32]))
                v.scalar_tensor_tensor(ifv, x0v[:, :, 1:2, :],
                                       float(W), basev, OP.mult, OP.add)
                v.tensor_tensor(iv, ifv, x0v[:, :, 0:1, :], OP.add)

            def emit_idx(cv, a):
                emit_floor(cv, a, 0)
                emit_addr(a)

            def emit_weights(cv, a):
                """frac -> W4 m=1 (DVE); u/t = 1-frac -> W4 m=0 (ACT)."""
                x0v = X0[:, 0:a * 64].rearrange("p (a k f) -> p a k f",
                                                a=a, k=2)
                w4kv = W4[:, 0:a * 128].rearrange(
                    "p (a i k m) -> p a k i m", a=a, i=32, k=2, m=2)
                v.tensor_tensor(w4kv[:, :, :, :, 1:2], one1(cv), one1(x0v),
                                OP.subtract)
                nc.scalar.activation(w4kv[:, :, :, :, 0:1],
                                     w4kv[:, :, :, :, 1:2],
                                     AF.Copy, bias=1.0, scale=-1.0)

            def cand_coords_view():
                return (CAND[:].rearrange("p (a s) -> p a s", a=2)
                        [:, :, 0:64]
                        .rearrange("p a (k f) -> p a k f", k=2))

            def cand_score_view():
                return (CAND[:].rearrange("p (a s) -> p a s", a=2)
                        [:, :, 64:96])

            # ---- init: both host-precomputed gathers fire on the idx DMA
            emit_gather(0, 32, 0)      # initial coords
            emit_gather(32, 64, 128)   # first propagate (h|v)

            nc.vector.tensor_copy(BEST[:, 0:64], state[:, 0:64])
            nc.vector.tensor_copy(CAND[:, 0:64], state[:, 672:736])
            nc.vector.tensor_copy(CAND[:, 96:160], state[:, 736:800])

            emit_score(G[:, 0:128], state[:, 288:416],
                       BEST[:, 64:96].rearrange("p (a f) -> p a f", a=1), 1)
            emit_score(G[:, 128:384], state[:, 416:672], cand_score_view(), 2)

            def emit_accepts():
                v.tensor_tensor(UPD[:], b3(CAND[:, 64:96]), b3(BEST[:, 64:96]),
                                OP.is_gt)
                v.copy_predicated(BEST[:], UPD[:], CAND[:, 0:96])
                v.tensor_tensor(UPD[:], b3(CAND[:, 160:192]),
                                b3(BEST[:, 64:96]), OP.is_gt)
                v.copy_predicated(BEST[:], UPD[:], CAND[:, 96:192])

            emit_accepts()  # completes propagate(1,1)

            def propagate(dx, dy):
                # cand_v coords (vx|vy): partition shift on the idle PE via
                # an exact permutation matmul (1.0*x and 0.0*x products and
                # single-term sums are bit-exact even in decomposed fp32)
                pv = PERM[:, 0:128] if dy == 1 else PERM[:, 128:256]
                nc.tensor.matmul(PS[:], pv, BEST[:, 0:64],
                                 start=True, stop=True)

                # cand_h coords (hx|hy): col-roll by dx on Pool (parallel
                # with the PE v-roll)
                dv2 = CAND[:, 0:64].rearrange("p (c f) -> p c f", c=2)
                sv2 = BEST[:, 0:64].rearrange("p (c f) -> p c f", c=2)
                cp = nc.gpsimd.tensor_copy
                if dx == 1:
                    cp(dv2[:, :, 1:32], sv2[:, :, 0:31])
                    cp(dv2[64:128, :, 0:1], sv2[0:64, :, 31:32])
                    cp(dv2[0:64, :, 0:1], sv2[64:128, :, 31:32])
                else:
                    cp(dv2[:, :, 0:31], sv2[:, :, 1:32])
                    cp(dv2[0:64, :, 31:32], sv2[64:128, :, 0:1])
                    cp(dv2[64:128, :, 31:32], sv2[0:64, :, 0:1])
                if dx == 1:
                    v.tensor_scalar(CAND[:, 0:32], CAND[:, 0:32], 1.0,
                                    float(W - 1), OP.add, OP.min)
                else:
                    v.tensor_scalar(CAND[:, 0:32], CAND[:, 0:32], -1.0, 0.0,
                                    OP.add, OP.max)
                # h floor on DVE fills the PE-matmul + sem latency
                emit_floor(CAND[:, 0:64].rearrange("p (a k f) -> p a k f",
                                                   a=1, k=2), 1, 0)

                # v coords from PSUM, then v floor
                v.tensor_copy(CAND[:, 96:128], PS[:, 0:32])
                if dy == 1:
                    v.tensor_scalar(CAND[:, 128:160], PS[:, 32:64], 1.0,
                                    float(H - 1), OP.add, OP.min)
                else:
                    v.tensor_scalar(CAND[:, 128:160], PS[:, 32:64], -1.0,
                                    0.0, OP.add, OP.max)
                emit_floor(CAND[:, 96:160].rearrange("p (a k f) -> p a k f",
                                                     a=1, k=2), 1, 64)
                emit_addr(2)
                emit_gather_I(64, 128)
                emit_weights(cand_coords_view(), 2)
                emit_score(G[:, 128:384], W4[:, 0:256], cand_score_view(), 2)
                emit_accepts()

            def random_search(k):
                v.tensor_tensor(RC[:, 0:64], BEST[:, 0:64], noise_view(k),
                                OP.add)
                v.tensor_scalar(RC[:, 0:64], RC[:, 0:64], 0.0, float(W - 1),
                                OP.max, OP.min)
                cv = RC[:, 0:64].rearrange("p (a k f) -> p a k f", a=1, k=2)
                emit_idx(cv, 1)
                emit_gather_I(32, 0)
                emit_weights(cv, 1)
                emit_score(G[:, 0:128], W4[:, 0:128],
                           RC[:, 64:96].rearrange("p (a f) -> p a f", a=1), 1)
                v.tensor_tensor(UPD[:], b3(RC[:, 64:96]), b3(BEST[:, 64:96]),
                                OP.is_gt)
                v.copy_predicated(BEST[:], UPD[:], RC[:])

            random_search(0)
            propagate(-1, -1)
            random_search(1)
            propagate(-1, 1)
            random_search(2)
            propagate(1, -1)

            nc.sync.dma_start(
                out_xy.ap().rearrange("n p f -> p n f"),
                BEST[:, 0:64].rearrange("p (n f) -> p n f", n=2),
            )

    nc.compile()
    return nc


def _get_program():
    if "nc" not in _CACHE:
        _CACHE["nc"] = _build_program()
    return _CACHE["nc"]


# ----------------------------------------------------------------------------
# Host-side helpers
# ----------------------------------------------------------------------------

def _to_layout(v):
    """[64(i), 64(j)] -> [128, 32]; partition = 64*(j//32)+i, free = j%32."""
    return np.ascontiguousarray(
        v.reshape(64, 2, 32).transpose(1, 0, 2).reshape(128, 32))


def _from_layout(a):
    """[128, 32] -> [64(i), 64(j)]."""
    return a.reshape(2, 64, 32).transpose(1, 0, 2).reshape(64, 64)


def _noise_arrays():
    """Mirror the reference's jax.random usage exactly, in-process."""
    import jax
    import jax.numpy as jnp

    key = jax.random.key(42)
    kf, kb = jax.random.split(key)
    out = []
    for kdir in (kf, kb):
        ks = jax.random.split(kdir, 3)
        out.append([np.asarray(R * jax.random.normal(k, (B, H, W, 2),
                                                     jnp.float32))
                    for k in ks])
    return out  # [dir][step] -> [B,H,W,2] float32


_PIX_BASE = None


def _pix_base():
    global _PIX_BASE
    if _PIX_BASE is None:
        _PIX_BASE = ((np.arange(64, dtype=np.int64)[:, None] * 64
                      + np.arange(64, dtype=np.int64)[None, :]) * PIX)
    return _PIX_BASE


def _quad_wi(x_plane, y_plane):
    """Clamped-floor weights + quad index for coords planes (fp32 mirror of
    the device arithmetic). Returns (W4 [128,128], idx [128,32] int32)."""
    one = np.float32(1.0)
    x0 = np.minimum(np.floor(x_plane), np.float32(W - 2)).astype(np.float32)
    y0 = np.minimum(np.floor(y_plane), np.float32(H - 2)).astype(np.float32)
    wx = x_plane - x0
    wy = y_plane - y0
    u = one - wx
    t = one - wy
    w4 = np.stack([_to_layout(u), _to_layout(wx),
                   _to_layout(t), _to_layout(wy)], axis=-1).reshape(128, 128)
    idx = (_pix_base()
           + y0.astype(np.int64) * 64 + x0.astype(np.int64))
    return np.ascontiguousarray(w4.astype(np.float32)), \
        _to_layout(idx).astype(np.int32)


def _clamp_np(x, lo, hi):
    return np.minimum(np.maximum(x, np.float32(lo)), np.float32(hi))


def _make_unit_inputs(x_plane, y_plane, noise_steps, b):
    """Build state [25,128,32] f32 and idx [128,96] i32 for one unit."""
    x_plane = x_plane.astype(np.float32)
    y_plane = y_plane.astype(np.float32)

    base_l = _to_layout(_pix_base().astype(np.float32))
    rows = [_to_layout(x_plane), _to_layout(y_plane), base_l]
    for step in range(3):
        nz = noise_steps[step][b]  # [H,W,2]
        rows.append(_to_layout(np.ascontiguousarray(nz[:, :, 0])))
        rows.append(_to_layout(np.ascontiguousarray(nz[:, :, 1])))

    w40, i0 = _quad_wi(x_plane, y_plane)
    rows.extend(w40.reshape(128, 4, 32).transpose(1, 0, 2))

    # first propagate (dx=1, dy=1) candidates, host-computed
    xh = _clamp_np(np.roll(x_plane, 1, axis=1) + np.float32(1.0), 0, W - 1)
    yh = _clamp_np(np.roll(y_plane, 1, axis=1), 0, H - 1)
    xv = _clamp_np(np.roll(x_plane, 1, axis=0), 0, W - 1)
    yv = _clamp_np(np.roll(y_plane, 1, axis=0) + np.float32(1.0), 0, H - 1)
    w4h, ih = _quad_wi(xh, yh)
    w4v, iv = _quad_wi(xv, yv)
    rows.extend(w4h.reshape(128, 4, 32).transpose(1, 0, 2))
    rows.extend(w4v.reshape(128, 4, 32).transpose(1, 0, 2))
    rows.extend([_to_layout(xh), _to_layout(yh),
                 _to_layout(xv), _to_layout(yv)])

    state = np.ascontiguousarray(np.stack(rows)).astype(np.float32)
    idx = np.ascontiguousarray(
        np.concatenate([i0, ih, iv], axis=1)).astype(np.int32)
    return state, idx


def _perm_mats():
    """lhsT matrices for the PE v-roll: out[m] = in[src(m)], src within
    each 64-partition block with wraparound. [0]=dy+1, [1]=dy-1."""
    P = np.zeros((2, 128, 128), np.float32)
    for m in range(128):
        blk = (m // 64) * 64
        P[0, blk + ((m - blk - 1) % 64), m] = 1.0
        P[1, blk + ((m - blk + 1) % 64), m] = 1.0
    return P


def _quad_corr(c):
    """c [4096, 64, 64] f32 -> quad [4096, 64, 64, 4]: per (y0, x0) the
    bilinear corner block [v00, v01, v10, v11] (edges padded by clamping;
    never addressed because x0, y0 <= 62)."""
    q = np.empty((PIX, H, W, 4), np.float32)
    q[..., 0] = c
    q[:, :, :-1, 1] = c[:, :, 1:]
    q[:, :, -1, 1] = c[:, :, -1]
    q[:, :-1, :, 2] = c[:, 1:, :]
    q[:, -1, :, 2] = c[:, -1, :]
    q[:, :-1, :-1, 3] = c[:, 1:, 1:]
    q[:, -1, :, 3] = q[:, -1, :, 1]
    q[:, :, -1, 3] = q[:, :, -1, 2]
    return q


def _bilinear_map_np(img, coords):
    """numpy mirror of reference._bilinear_map (fp32, same op order)."""
    Bn, Hn, Wn, C = img.shape
    out = np.empty_like(img)
    one = np.float32(1.0)
    for b in range(Bn):
        x = coords[b, :, :, 0].reshape(-1)
        y = coords[b, :, :, 1].reshape(-1)
        x0 = np.floor(x)
        y0 = np.floor(y)
        wx = (x - x0)[:, None]
        wy = (y - y0)[:, None]
        x0i = np.clip(x0.astype(np.int32), 0, Wn - 1)
        x1i = np.clip(x0i + 1, 0, Wn - 1)
        y0i = np.clip(y0.astype(np.int32), 0, Hn - 1)
        y1i = np.clip(y0i + 1, 0, Hn - 1)
        im = img[b]
        v00 = im[y0i, x0i]
        v01 = im[y0i, x1i]
        v10 = im[y1i, x0i]
        v11 = im[y1i, x1i]
        o = (v00 * (one - wx) * (one - wy) + v01 * wx * (one - wy)
             + v10 * (one - wx) * wy + v11 * wx * wy)
        out[b] = o.reshape(Hn, Wn, C)
    return out


def _run_device(in_maps, trace=False):
    from concourse import bass_utils

    nc = _get_program()
    res = bass_utils.run_bass_kernel_spmd(
        nc, in_maps, core_ids=list(range(N_CORES)), trace=trace)
    return res


def kernel(matching_f, matching_b, corr_map, _trace=False, _results_hook=None):
    matching_f = np.asarray(matching_f)
    matching_b = np.asarray(matching_b)
    corr_map = np.asarray(corr_map)

    noise = _noise_arrays()  # [dir][step][B,H,W,2]
    perm = _perm_mats()

    in_maps = []
    for b in range(B):  # forward units, cores 0..3
        cq = _quad_corr(np.ascontiguousarray(corr_map[b]).reshape(
            PIX, H, W))
        state, idx = _make_unit_inputs(matching_f[b, 0], matching_f[b, 1],
                                       noise[0], b)
        in_maps.append({"corr": cq.reshape(PIX * PIX, 4),
                        "state": state, "idx": idx, "perm": perm})
    for b in range(B):  # backward units, cores 4..7
        corr_t = np.ascontiguousarray(corr_map[b].transpose(2, 3, 0, 1))
        cq = _quad_corr(corr_t.reshape(PIX, H, W))
        state, idx = _make_unit_inputs(matching_b[b, 0], matching_b[b, 1],
                                       noise[1], b)
        in_maps.append({"corr": cq.reshape(PIX * PIX, 4),
                        "state": state, "idx": idx, "perm": perm})

    res = _run_device(in_maps, trace=_trace)
    if _results_hook is not None:
        _results_hook(res)

    res_f = np.empty((B, H, W, 2), np.float32)
    res_b = np.empty((B, H, W, 2), np.float32)
    for b in range(B):
        of = res.results[b]["out_xy"]
        ob = res.results[4 + b]["out_xy"]
        res_f[b, :, :, 0] = _from_layout(of[0])
        res_f[b, :, :, 1] = _from_layout(of[1])
        res_b[b, :, :, 0] = _from_layout(ob[0])
        res_b[b, :, :, 1] = _from_layout(ob[1])

    # forward-backward consistency (host; mirrors reference in fp32)
    counter = _bilinear_map_np(res_b, res_f)
    diff = np.max(np.abs(res_f - counter), axis=-1)
    invalid = (diff > EPS)[..., None]
    mf_t = matching_f.transpose(0, 2, 3, 1)  # [B,H,W,2]
    out = np.where(invalid, mf_t, res_f)
    return np.ascontiguousarray(out.transpose(0, 3, 1, 2)).astype(np.float32)


# revision 25
# speedup vs baseline: 1.0888x; 1.0888x over previous
"""PatchMatch-style MatchingPropagator on 8 Trainium2 NeuronCores.

Full inputs in, full outputs out. Sharding: 8 independent units =
(direction in {forward, backward}) x (batch 0..3), one NeuronCore each.
Core b runs forward for batch b; core 4+b runs backward for batch b using
the host-transposed correlation volume.

v2: the correlation volume is expanded on the host into a "quad" layout
corr_q[pixel, y0, x0] = [v00, v01, v10, v11] so one bilinear sample is a
single 16-byte indirect-DMA descriptor (half the SWDGE descriptor-
generation work, which is the dominant serial cost). The initial eval and
the first propagate depend only on the inputs, so their gather indices and
weight quads are precomputed on the host and shipped with the state; both
gathers issue as soon as the small index DMA lands. Scores are computed in
5 wide DVE ops via broadcast views of interleaved weight quads
[u, wx, t, wy], keeping the reference's exact fp32 multiply/add tree so
every argmax decision matches the reference bitwise. The gather address
chain runs on the Pool engine (which also issues the gather).

Pixel layout on chip: pixel (i, j) -> partition 64*(j//32) + i, free j%32.
"""

import numpy as np

B, H, W = 4, 64, 64
R = 3.0
EPS = np.float32(0.01)
N_CORES = 8
PIX = H * W  # 4096 pixels per unit; each owns a 64x64 correlation map
M_RNE = float(1 << 23)

_CACHE = {}


# ----------------------------------------------------------------------------
# Device program (SPMD: identical on all 8 cores; data differs per core)
# ----------------------------------------------------------------------------

def _build_program():
    import concourse.bass as bass
    import concourse.mybir as mybir
    import concourse.tile as tile
    from concourse import bacc

    F32 = mybir.dt.float32
    BF16 = mybir.dt.bfloat16
    I32 = mybir.dt.int32
    OP = mybir.AluOpType
    AF = mybir.ActivationFunctionType

    nc = bacc.Bacc(
        "TRN2",
        target_bir_lowering=False,
        debug=False,
        enable_asserts=False,
        num_devices=N_CORES,
    )

    # quad corr: row r = pixel*4096 + y0*64 + x0 -> [v00, v01, v10, v11]
    corr = nc.dram_tensor("corr", [PIX * PIX, 4], F32, kind="ExternalInput")
    # state rows (partition-major so the DMA is one contiguous run per
    # partition): 0 x, 1 y, 2 base, 3..8 noise (nx,ny)*3, 9..12 W40,
    # 13..20 W41, 21..22 cand_h xy, 23..24 cand_v xy
    state_in = nc.dram_tensor("state", [128, 25 * 32], F32,
                              kind="ExternalInput")
    idx_in = nc.dram_tensor("idx", [128, 96], I32, kind="ExternalInput")
    # partition-shift permutation matrices for the v-roll (dy=+1, dy=-1)
    perm_in = nc.dram_tensor("perm", [128, 256], F32,
                             kind="ExternalInput")
    out_xy = nc.dram_tensor("out_xy", [2, 128, 32], F32,
                            kind="ExternalOutput")

    corr_ap = corr.ap()

    def b3(ap):  # [128,32] -> broadcast [128,3,32]
        return ap.rearrange("p (one f) -> p one f", one=1).to_broadcast(
            [128, 3, 32])

    with tile.TileContext(nc) as tc:
        with tc.tile_pool(name="main", bufs=1) as pool, \
                tc.tile_pool(name="ps", bufs=1,
                             space=bass.MemorySpace.PSUM) as ppool:
            IDX = pool.tile([128, 96], I32, name="IDX")
            state = pool.tile([128, 25 * 32], F32, name="state")
            PERM = pool.tile([128, 256], F32, name="PERM")
            PS = ppool.tile([128, 64], F32, name="PS")
            nc.sync.dma_start(IDX[:], idx_in.ap())
            nc.sync.dma_start(state[:], state_in.ap())
            nc.sync.dma_start(PERM[:], perm_in.ap())
            base_row = state[:, 64:96]

            def noise_view(k):
                o = 96 + 64 * k
                return state[:, o:o + 64]  # [nx|ny]

            BEST = pool.tile([128, 96], F32, name="BEST")    # [x|y|s]
            CAND = pool.tile([128, 192], F32, name="CAND")   # [hx hy hs|vx vy vs]
            RC = pool.tile([128, 96], F32, name="RC")        # [x y s]
            WF = pool.tile([128, 128], F32, name="WF")
            X0 = pool.tile([128, 128], F32, name="X0")
            W4 = pool.tile([128, 256], F32, name="W4")       # [u wx t wy]*
            IF = pool.tile([128, 64], F32, name="IF")
            I = pool.tile([128, 64], I32, name="I")
            G = pool.tile([128, 384], F32, name="G")         # single|dual
            At = pool.tile([128, 256], F32, name="At")
            Bt = pool.tile([128, 256], F32, name="Bt")
            D1 = pool.tile([128, 64], F32, name="D1")
            D2 = pool.tile([128, 64], F32, name="D2")
            UPD = pool.tile([128, 96], I32, name="UPD")

            v = nc.vector

            def emit_gather(icols_off, n_idx, gcols_off):
                nc.gpsimd.indirect_dma_start(
                    out=G[:, gcols_off:gcols_off + 4 * n_idx],
                    out_offset=None,
                    in_=corr_ap,
                    in_offset=bass.IndirectOffsetOnAxis(
                        ap=IDX[:, icols_off:icols_off + n_idx], axis=0),
                )

            def emit_gather_I(n_idx, gcols_off):
                nc.gpsimd.indirect_dma_start(
                    out=G[:, gcols_off:gcols_off + 4 * n_idx],
                    out_offset=None,
                    in_=corr_ap,
                    in_offset=bass.IndirectOffsetOnAxis(
                        ap=I[:, 0:n_idx], axis=0),
                )

            def one1(ap):  # append a trailing size-1 dim
                return ap.rearrange("... (f one) -> ... f one", one=1)

            def emit_score(g_flat, w_flat, out_v, a, eng=None, scr=0):
                """bilinear scores for `a` candidates: 5 ops, exact fp32
                tree ((v00*u)*t + (v01*wx)*t + (v10*u)*wy) + (v11*wx)*wy
                evaluated left-to-right like the reference. `scr` picks a
                scratch half so two engines can score concurrently."""
                e = eng or v
                g = g_flat.rearrange("p (a i r c) -> p a i r c",
                                     a=a, i=32, r=2, c=2)
                w4v = w_flat.rearrange("p (a i k m) -> p a i k m",
                                       a=a, i=32, k=2, m=2)
                uw = w4v[:, :, :, 0:1, :].to_broadcast([128, a, 32, 2, 2])
                twy = (w4v[:, :, :, 1:2, :]
                       .rearrange("p a i k m -> p a i m k")
                       .to_broadcast([128, a, 32, 2, 2]))
                ao, do = 128 * scr, 32 * scr
                atv = At[:, ao:ao + a * 128].rearrange(
                    "p (a i r c) -> p a i r c", a=a, i=32, r=2, c=2)
                btv = Bt[:, ao:ao + a * 128].rearrange(
                    "p (a i r c) -> p a i r c", a=a, i=32, r=2, c=2)
                e.tensor_tensor(atv, g, uw, OP.mult)
                e.tensor_tensor(btv, atv, twy, OP.mult)
                q = Bt[:, ao:ao + a * 128].rearrange(
                    "p (a i q) -> p a i q", a=a, i=32, q=4)
                d1 = one1(D1[:, do:do + a * 32]
                          .rearrange("p (a f) -> p a f", a=a))
                d2 = one1(D2[:, do:do + a * 32]
                          .rearrange("p (a f) -> p a f", a=a))
                e.tensor_tensor(d1, q[:, :, :, 0:1], q[:, :, :, 1:2], OP.add)
                e.tensor_tensor(d2, d1, q[:, :, :, 2:3], OP.add)
                e.tensor_tensor(one1(out_v), d2, q[:, :, :, 3:4], OP.add)

            def emit_floor(cv, a, off):
                """cv: coords view [128, a, 2(k=x/y), 32]. Clamped floor ->
                X0[:, off:off+a*64]."""
                wfv = WF[:, off:off + a * 64].rearrange(
                    "p (a k f) -> p a k f", a=a, k=2)
                x0v = X0[:, off:off + a * 64].rearrange(
                    "p (a k f) -> p a k f", a=a, k=2)
                v.tensor_scalar(wfv, cv, M_RNE, M_RNE, OP.add, OP.subtract)
                v.tensor_tensor(x0v, wfv, cv, OP.is_gt)
                v.tensor_tensor(x0v, wfv, x0v, OP.subtract)
                v.tensor_scalar(x0v, x0v, float(W - 2), None, OP.min)

            def emit_addr(a):
                """int32 quad indices -> I from X0[:, 0:a*64]."""
                x0v = X0[:, 0:a * 64].rearrange("p (a k f) -> p a k f",
                                                a=a, k=2)
                ifv = IF[:, 0:a * 32].rearrange("p (a one f) -> p a one f",
                                                a=a, one=1)
                iv = I[:, 0:a * 32].rearrange("p (a one f) -> p a one f",
                                              a=a, one=1)
                basev = (base_row.rearrange("p (one onee f) -> p one onee f",
                                            one=1, onee=1)
                         .to_broadcast([128, a, 1, 32]))
                v.scalar_tensor_tensor(ifv, x0v[:, :, 1:2, :],
                                       float(W), basev, OP.mult, OP.add)
                v.tensor_tensor(iv, ifv, x0v[:, :, 0:1, :], OP.add)

            def emit_idx(cv, a):
                emit_floor(cv, a, 0)
                emit_addr(a)

            def emit_weights(cv, a):
                """frac -> W4 m=1 (DVE); u/t = 1-frac -> W4 m=0 (ACT)."""
                x0v = X0[:, 0:a * 64].rearrange("p (a k f) -> p a k f",
                                                a=a, k=2)
                w4kv = W4[:, 0:a * 128].rearrange(
                    "p (a i k m) -> p a k i m", a=a, i=32, k=2, m=2)
                v.tensor_tensor(w4kv[:, :, :, :, 1:2], one1(cv), one1(x0v),
                                OP.subtract)
                nc.scalar.activation(w4kv[:, :, :, :, 0:1],
                                     w4kv[:, :, :, :, 1:2],
                                     AF.Copy, bias=1.0, scale=-1.0)

            def cand_coords_view():
                return (CAND[:].rearrange("p (a s) -> p a s", a=2)
                        [:, :, 0:64]
                        .rearrange("p a (k f) -> p a k f", k=2))

            def cand_score_view():
                return (CAND[:].rearrange("p (a s) -> p a s", a=2)
                        [:, :, 64:96])

            # ---- init: both host-precomputed gathers fire on the idx DMA
            emit_gather(0, 32, 0)      # initial coords
            emit_gather(32, 64, 128)   # first propagate (h|v)

            nc.vector.tensor_copy(BEST[:, 0:64], state[:, 0:64])
            nc.vector.tensor_copy(CAND[:, 0:64], state[:, 672:736])
            nc.vector.tensor_copy(CAND[:, 96:160], state[:, 736:800])

            emit_score(G[:, 0:128], state[:, 288:416],
                       BEST[:, 64:96].rearrange("p (a f) -> p a f", a=1), 1)
            emit_score(G[:, 128:384], state[:, 416:672], cand_score_view(), 2)

            def emit_accepts():
                v.tensor_tensor(UPD[:], b3(CAND[:, 64:96]), b3(BEST[:, 64:96]),
                                OP.is_gt)
                v.copy_predicated(BEST[:], UPD[:], CAND[:, 0:96])
                v.tensor_tensor(UPD[:], b3(CAND[:, 160:192]),
                                b3(BEST[:, 64:96]), OP.is_gt)
                v.copy_predicated(BEST[:], UPD[:], CAND[:, 96:192])

            emit_accepts()  # completes propagate(1,1)

            def propagate(dx, dy):
                # cand_v coords (vx|vy): partition shift on the idle PE via
                # an exact permutation matmul (1.0*x and 0.0*x products and
                # single-term sums are bit-exact even in decomposed fp32)
                pv = PERM[:, 0:128] if dy == 1 else PERM[:, 128:256]
                nc.tensor.matmul(PS[:], pv, BEST[:, 0:64],
                                 start=True, stop=True)

                # cand_h coords (hx|hy): col-roll by dx on Pool (parallel
                # with the PE v-roll)
                dv2 = CAND[:, 0:64].rearrange("p (c f) -> p c f", c=2)
                sv2 = BEST[:, 0:64].rearrange("p (c f) -> p c f", c=2)
                cp = nc.gpsimd.tensor_copy
                if dx == 1:
                    cp(dv2[:, :, 1:32], sv2[:, :, 0:31])
                    cp(dv2[64:128, :, 0:1], sv2[0:64, :, 31:32])
                    cp(dv2[0:64, :, 0:1], sv2[64:128, :, 31:32])
                else:
                    cp(dv2[:, :, 0:31], sv2[:, :, 1:32])
                    cp(dv2[0:64, :, 31:32], sv2[64:128, :, 0:1])
                    cp(dv2[64:128, :, 31:32], sv2[0:64, :, 0:1])
                if dx == 1:
                    v.tensor_scalar(CAND[:, 0:32], CAND[:, 0:32], 1.0,
                                    float(W - 1), OP.add, OP.min)
                else:
                    v.tensor_scalar(CAND[:, 0:32], CAND[:, 0:32], -1.0, 0.0,
                                    OP.add, OP.max)
                # h floor on DVE fills the PE-matmul + sem latency
                emit_floor(CAND[:, 0:64].rearrange("p (a k f) -> p a k f",
                                                   a=1, k=2), 1, 0)

                # v coords from PSUM, then v floor
                v.tensor_copy(CAND[:, 96:128], PS[:, 0:32])
                if dy == 1:
                    v.tensor_scalar(CAND[:, 128:160], PS[:, 32:64], 1.0,
                                    float(H - 1), OP.add, OP.min)
                else:
                    v.tensor_scalar(CAND[:, 128:160], PS[:, 32:64], -1.0,
                                    0.0, OP.add, OP.max)
                emit_floor(CAND[:, 96:160].rearrange("p (a k f) -> p a k f",
                                                     a=1, k=2), 1, 64)
                emit_addr(2)
                emit_gather_I(64, 128)
                emit_weights(cand_coords_view(), 2)
                emit_score(G[:, 128:384], W4[:, 0:256], cand_score_view(), 2)
                emit_accepts()

            def random_search(k):
                v.tensor_tensor(RC[:, 0:64], BEST[:, 0:64], noise_view(k),
                                OP.add)
                v.tensor_scalar(RC[:, 0:64], RC[:, 0:64], 0.0, float(W - 1),
                                OP.max, OP.min)
                cv = RC[:, 0:64].rearrange("p (a k f) -> p a k f", a=1, k=2)
                emit_idx(cv, 1)
                emit_gather_I(32, 0)
                emit_weights(cv, 1)
                emit_score(G[:, 0:128], W4[:, 0:128],
                           RC[:, 64:96].rearrange("p (a f) -> p a f", a=1), 1)
                v.tensor_tensor(UPD[:], b3(RC[:, 64:96]), b3(BEST[:, 64:96]),
                                OP.is_gt)
                v.copy_predicated(BEST[:], UPD[:], RC[:])

            random_search(0)
            propagate(-1, -1)
            random_search(1)
            propagate(-1, 1)
            random_search(2)
            propagate(1, -1)

            nc.sync.dma_start(
                out_xy.ap().rearrange("n p f -> p n f"),
                BEST[:, 0:64].rearrange("p (n f) -> p n f", n=2),
            )

    nc.compile()
    return nc


def _get_program():
    if "nc" not in _CACHE:
        _CACHE["nc"] = _build_program()
    return _CACHE["nc"]


# ----------------------------------------------------------------------------
# Host-side helpers
# ----------------------------------------------------------------------------

def _to_layout(v):
    """[64(i), 64(j)] -> [128, 32]; partition = 64*(j//32)+i, free = j%32."""
    return np.ascontiguousarray(
        v.reshape(64, 2, 32).transpose(1, 0, 2).reshape(128, 32))


def _from_layout(a):
    """[128, 32] -> [64(i), 64(j)]."""
    return a.reshape(2, 64, 32).transpose(1, 0, 2).reshape(64, 64)


def _noise_arrays():
    """Mirror the reference's jax.random usage exactly, in-process."""
    import jax
    import jax.numpy as jnp

    key = jax.random.key(42)
    kf, kb = jax.random.split(key)
    out = []
    for kdir in (kf, kb):
        ks = jax.random.split(kdir, 3)
        out.append([np.asarray(R * jax.random.normal(k, (B, H, W, 2),
                                                     jnp.float32))
                    for k in ks])
    return out  # [dir][step] -> [B,H,W,2] float32


_PIX_BASE = None


def _pix_base():
    global _PIX_BASE
    if _PIX_BASE is None:
        _PIX_BASE = ((np.arange(64, dtype=np.int64)[:, None] * 64
                      + np.arange(64, dtype=np.int64)[None, :]) * PIX)
    return _PIX_BASE


def _quad_wi(x_plane, y_plane):
    """Clamped-floor weights + quad index for coords planes (fp32 mirror of
    the device arithmetic). Returns (W4 [128,128], idx [128,32] int32)."""
    one = np.float32(1.0)
    x0 = np.minimum(np.floor(x_plane), np.float32(W - 2)).astype(np.float32)
    y0 = np.minimum(np.floor(y_plane), np.float32(H - 2)).astype(np.float32)
    wx = x_plane - x0
    wy = y_plane - y0
    u = one - wx
    t = one - wy
    w4 = np.stack([_to_layout(u), _to_layout(wx),
                   _to_layout(t), _to_layout(wy)], axis=-1).reshape(128, 128)
    idx = (_pix_base()
           + y0.astype(np.int64) * 64 + x0.astype(np.int64))
    return np.ascontiguousarray(w4.astype(np.float32)), \
        _to_layout(idx).astype(np.int32)


def _clamp_np(x, lo, hi):
    return np.minimum(np.maximum(x, np.float32(lo)), np.float32(hi))


def _make_unit_inputs(x_plane, y_plane, noise_steps, b):
    """Build state [25,128,32] f32 and idx [128,96] i32 for one unit."""
    x_plane = x_plane.astype(np.float32)
    y_plane = y_plane.astype(np.float32)

    base_l = _to_layout(_pix_base().astype(np.float32))
    rows = [_to_layout(x_plane), _to_layout(y_plane), base_l]
    for step in range(3):
        nz = noise_steps[step][b]  # [H,W,2]
        rows.append(_to_layout(np.ascontiguousarray(nz[:, :, 0])))
        rows.append(_to_layout(np.ascontiguousarray(nz[:, :, 1])))

    w40, i0 = _quad_wi(x_plane, y_plane)
    rows.extend(w40.reshape(128, 4, 32).transpose(1, 0, 2))

    # first propagate (dx=1, dy=1) candidates, host-computed
    xh = _clamp_np(np.roll(x_plane, 1, axis=1) + np.float32(1.0), 0, W - 1)
    yh = _clamp_np(np.roll(y_plane, 1, axis=1), 0, H - 1)
    xv = _clamp_np(np.roll(x_plane, 1, axis=0), 0, W - 1)
    yv = _clamp_np(np.roll(y_plane, 1, axis=0) + np.float32(1.0), 0, H - 1)
    w4h, ih = _quad_wi(xh, yh)
    w4v, iv = _quad_wi(xv, yv)
    rows.extend(w4h.reshape(128, 4, 32).transpose(1, 0, 2))
    rows.extend(w4v.reshape(128, 4, 32).transpose(1, 0, 2))
    rows.extend([_to_layout(xh), _to_layout(yh),
                 _to_layout(xv), _to_layout(yv)])

    state = np.stack(rows).astype(np.float32)  # [25,128,32]
    state = np.ascontiguousarray(
        state.transpose(1, 0, 2).reshape(128, 25 * 32))
    idx = np.ascontiguousarray(
        np.concatenate([i0, ih, iv], axis=1)).astype(np.int32)
    return state, idx


def _perm_mats():
    """lhsT matrices for the PE v-roll: out[m] = in[src(m)], src within
    each 64-partition block with wraparound. [0]=dy+1, [1]=dy-1."""
    P = np.zeros((2, 128, 128), np.float32)
    for m in range(128):
        blk = (m // 64) * 64
        P[0, blk + ((m - blk - 1) % 64), m] = 1.0
        P[1, blk + ((m - blk + 1) % 64), m] = 1.0
    return np.ascontiguousarray(P.transpose(1, 0, 2).reshape(128, 256))


def _quad_corr(c):
    """c [4096, 64, 64] f32 -> quad [4096, 64, 64, 4]: per (y0, x0) the
    bilinear corner block [v00, v01, v10, v11] (edges padded by clamping;
    never addressed because x0, y0 <= 62)."""
    q = np.empty((PIX, H, W, 4), np.float32)
    q[..., 0] = c
    q[:, :, :-1, 1] = c[:, :, 1:]
    q[:, :, -1, 1] = c[:, :, -1]
    q[:, :-1, :, 2] = c[:, 1:, :]
    q[:, -1, :, 2] = c[:, -1, :]
    q[:, :-1, :-1, 3] = c[:, 1:, 1:]
    q[:, -1, :, 3] = q[:, -1, :, 1]
    q[:, :, -1, 3] = q[:, :, -1, 2]
    return q


def _bilinear_map_np(img, coords):
    """numpy mirror of reference._bilinear_map (fp32, same op order)."""
    Bn, Hn, Wn, C = img.shape
    out = np.empty_like(img)
    one = np.float32(1.0)
    for b in range(Bn):
        x = coords[b, :, :, 0].reshape(-1)
        y = coords[b, :, :, 1].reshape(-1)
        x0 = np.floor(x)
        y0 = np.floor(y)
        wx = (x - x0)[:, None]
        wy = (y - y0)[:, None]
        x0i = np.clip(x0.astype(np.int32), 0, Wn - 1)
        x1i = np.clip(x0i + 1, 0, Wn - 1)
        y0i = np.clip(y0.astype(np.int32), 0, Hn - 1)
        y1i = np.clip(y0i + 1, 0, Hn - 1)
        im = img[b]
        v00 = im[y0i, x0i]
        v01 = im[y0i, x1i]
        v10 = im[y1i, x0i]
        v11 = im[y1i, x1i]
        o = (v00 * (one - wx) * (one - wy) + v01 * wx * (one - wy)
             + v10 * (one - wx) * wy + v11 * wx * wy)
        out[b] = o.reshape(Hn, Wn, C)
    return out


def _run_device(in_maps, trace=False):
    from concourse import bass_utils

    nc = _get_program()
    res = bass_utils.run_bass_kernel_spmd(
        nc, in_maps, core_ids=list(range(N_CORES)), trace=trace)
    return res


def kernel(matching_f, matching_b, corr_map, _trace=False, _results_hook=None):
    matching_f = np.asarray(matching_f)
    matching_b = np.asarray(matching_b)
    corr_map = np.asarray(corr_map)

    noise = _noise_arrays()  # [dir][step][B,H,W,2]
    perm = _perm_mats()

    in_maps = []
    for b in range(B):  # forward units, cores 0..3
        cq = _quad_corr(np.ascontiguousarray(corr_map[b]).reshape(
            PIX, H, W))
        state, idx = _make_unit_inputs(matching_f[b, 0], matching_f[b, 1],
                                       noise[0], b)
        in_maps.append({"corr": cq.reshape(PIX * PIX, 4),
                        "state": state, "idx": idx, "perm": perm})
    for b in range(B):  # backward units, cores 4..7
        corr_t = np.ascontiguousarray(corr_map[b].transpose(2, 3, 0, 1))
        cq = _quad_corr(corr_t.reshape(PIX, H, W))
        state, idx = _make_unit_inputs(matching_b[b, 0], matching_b[b, 1],
                                       noise[1], b)
        in_maps.append({"corr": cq.reshape(PIX * PIX, 4),
                        "state": state, "idx": idx, "perm": perm})

    res = _run_device(in_maps, trace=_trace)
    if _results_hook is not None:
        _results_hook(res)

    res_f = np.empty((B, H, W, 2), np.float32)
    res_b = np.empty((B, H, W, 2), np.float32)
    for b in range(B):
        of = res.results[b]["out_xy"]
        ob = res.results[4 + b]["out_xy"]
        res_f[b, :, :, 0] = _from_layout(of[0])
        res_f[b, :, :, 1] = _from_layout(of[1])
        res_b[b, :, :, 0] = _from_layout(ob[0])
        res_b[b, :, :, 1] = _from_layout(ob[1])

    # forward-backward consistency (host; mirrors reference in fp32)
    counter = _bilinear_map_np(res_b, res_f)
    diff = np.max(np.abs(res_f - counter), axis=-1)
    invalid = (diff > EPS)[..., None]
    mf_t = matching_f.transpose(0, 2, 3, 1)  # [B,H,W,2]
    out = np.where(invalid, mf_t, res_f)
    return np.ascontiguousarray(out.transpose(0, 3, 1, 2)).astype(np.float32)


# revision 29
# speedup vs baseline: 1.1028x; 1.0129x over previous
"""PatchMatch-style MatchingPropagator on 8 Trainium2 NeuronCores.

Full inputs in, full outputs out. Sharding: 8 independent units =
(direction in {forward, backward}) x (batch 0..3), one NeuronCore each.
Core b runs forward for batch b; core 4+b runs backward for batch b using
the host-transposed correlation volume.

v2: the correlation volume is expanded on the host into a "quad" layout
corr_q[pixel, y0, x0] = [v00, v01, v10, v11] so one bilinear sample is a
single 16-byte indirect-DMA descriptor (half the SWDGE descriptor-
generation work, which is the dominant serial cost). The initial eval and
the first propagate depend only on the inputs, so their gather indices and
weight quads are precomputed on the host and shipped with the state; both
gathers issue as soon as the small index DMA lands. Scores are computed in
5 wide DVE ops via broadcast views of interleaved weight quads
[u, wx, t, wy], keeping the reference's exact fp32 multiply/add tree so
every argmax decision matches the reference bitwise. The gather address
chain runs on the Pool engine (which also issues the gather).

Pixel layout on chip: pixel (i, j) -> partition 64*(j//32) + i, free j%32.
"""

import numpy as np

B, H, W = 4, 64, 64
R = 3.0
EPS = np.float32(0.01)
N_CORES = 8
PIX = H * W  # 4096 pixels per unit; each owns a 64x64 correlation map
M_RNE = float(1 << 23)

_CACHE = {}


# ----------------------------------------------------------------------------
# Device program (SPMD: identical on all 8 cores; data differs per core)
# ----------------------------------------------------------------------------

def _build_program():
    import concourse.bass as bass
    import concourse.mybir as mybir
    import concourse.tile as tile
    from concourse import bacc

    F32 = mybir.dt.float32
    BF16 = mybir.dt.bfloat16
    I32 = mybir.dt.int32
    OP = mybir.AluOpType
    AF = mybir.ActivationFunctionType

    nc = bacc.Bacc(
        "TRN2",
        target_bir_lowering=False,
        debug=False,
        enable_asserts=False,
        num_devices=N_CORES,
    )

    # quad corr: row r = pixel*4096 + y0*64 + x0 -> [v00, v01, v10, v11]
    corr = nc.dram_tensor("corr", [PIX * PIX, 4], F32, kind="ExternalInput")
    # state rows (partition-major so the DMA is one contiguous run per
    # partition): 0 x, 1 y, 2 base, 3..8 noise (nx,ny)*3, 9..12 W40,
    # 13..20 W41, 21..22 cand_h xy, 23..24 cand_v xy
    state_in = nc.dram_tensor("state", [128, 25 * 32], F32,
                              kind="ExternalInput")
    idx_in = nc.dram_tensor("idx", [128, 96], I32, kind="ExternalInput")
    # partition-shift permutation matrices for the v-roll (dy=+1, dy=-1)
    perm_in = nc.dram_tensor("perm", [128, 256], F32,
                             kind="ExternalInput")
    out_xy = nc.dram_tensor("out_xy", [128, 64], F32,
                            kind="ExternalOutput")

    corr_ap = corr.ap()

    def b3(ap):  # [128,32] -> broadcast [128,3,32]
        return ap.rearrange("p (one f) -> p one f", one=1).to_broadcast(
            [128, 3, 32])

    with tile.TileContext(nc) as tc:
        with tc.tile_pool(name="main", bufs=1) as pool, \
                tc.tile_pool(name="ps", bufs=1,
                             space=bass.MemorySpace.PSUM) as ppool:
            IDX = pool.tile([128, 96], I32, name="IDX")
            state = pool.tile([128, 25 * 32], F32, name="state")
            PERM = pool.tile([128, 256], F32, name="PERM")
            PS = ppool.tile([128, 64], F32, name="PS")
            nc.sync.dma_start(IDX[:], idx_in.ap())
            nc.sync.dma_start(state[:], state_in.ap())
            nc.sync.dma_start(PERM[:], perm_in.ap())
            base_row = state[:, 64:96]

            def noise_view(k):
                o = 96 + 64 * k
                return state[:, o:o + 64]  # [nx|ny]

            BEST = pool.tile([128, 96], F32, name="BEST")    # [x|y|s]
            CAND = pool.tile([128, 192], F32, name="CAND")   # [hx hy hs|vx vy vs]
            RC = pool.tile([128, 96], F32, name="RC")        # [x y s]
            WF = pool.tile([128, 128], F32, name="WF")
            X0 = pool.tile([128, 128], F32, name="X0")
            W4 = pool.tile([128, 256], F32, name="W4")       # [u wx t wy]*
            IF = pool.tile([128, 64], F32, name="IF")
            I = pool.tile([128, 64], I32, name="I")
            G = pool.tile([128, 384], F32, name="G")         # single|dual
            At = pool.tile([128, 256], F32, name="At")
            Bt = pool.tile([128, 256], F32, name="Bt")
            D1 = pool.tile([128, 64], F32, name="D1")
            D2 = pool.tile([128, 64], F32, name="D2")
            UPD = pool.tile([128, 96], I32, name="UPD")

            v = nc.vector

            def emit_gather(icols_off, n_idx, gcols_off):
                nc.gpsimd.indirect_dma_start(
                    out=G[:, gcols_off:gcols_off + 4 * n_idx],
                    out_offset=None,
                    in_=corr_ap,
                    in_offset=bass.IndirectOffsetOnAxis(
                        ap=IDX[:, icols_off:icols_off + n_idx], axis=0),
                )

            def emit_gather_I(n_idx, gcols_off):
                nc.gpsimd.indirect_dma_start(
                    out=G[:, gcols_off:gcols_off + 4 * n_idx],
                    out_offset=None,
                    in_=corr_ap,
                    in_offset=bass.IndirectOffsetOnAxis(
                        ap=I[:, 0:n_idx], axis=0),
                )

            def one1(ap):  # append a trailing size-1 dim
                return ap.rearrange("... (f one) -> ... f one", one=1)

            def emit_score(g_flat, w_flat, out_v, a, eng=None, scr=0):
                """bilinear scores for `a` candidates: 5 ops, exact fp32
                tree ((v00*u)*t + (v01*wx)*t + (v10*u)*wy) + (v11*wx)*wy
                evaluated left-to-right like the reference. `scr` picks a
                scratch half so two engines can score concurrently."""
                e = eng or v
                g = g_flat.rearrange("p (a i r c) -> p a i r c",
                                     a=a, i=32, r=2, c=2)
                w4v = w_flat.rearrange("p (a i k m) -> p a i k m",
                                       a=a, i=32, k=2, m=2)
                uw = w4v[:, :, :, 0:1, :].to_broadcast([128, a, 32, 2, 2])
                twy = (w4v[:, :, :, 1:2, :]
                       .rearrange("p a i k m -> p a i m k")
                       .to_broadcast([128, a, 32, 2, 2]))
                ao, do = 128 * scr, 32 * scr
                atv = At[:, ao:ao + a * 128].rearrange(
                    "p (a i r c) -> p a i r c", a=a, i=32, r=2, c=2)
                btv = Bt[:, ao:ao + a * 128].rearrange(
                    "p (a i r c) -> p a i r c", a=a, i=32, r=2, c=2)
                e.tensor_tensor(atv, g, uw, OP.mult)
                e.tensor_tensor(btv, atv, twy, OP.mult)
                q = Bt[:, ao:ao + a * 128].rearrange(
                    "p (a i q) -> p a i q", a=a, i=32, q=4)
                # innermost-axis add-reduce accumulates left-to-right,
                # matching the reference's ((q0+q1)+q2)+q3 exactly
                e.tensor_reduce(out_v, q, mybir.AxisListType.X, OP.add)

            def emit_floor(cv, a, off):
                """cv: coords view [128, a, 2(k=x/y), 32]. Clamped floor ->
                X0[:, off:off+a*64]."""
                wfv = WF[:, off:off + a * 64].rearrange(
                    "p (a k f) -> p a k f", a=a, k=2)
                x0v = X0[:, off:off + a * 64].rearrange(
                    "p (a k f) -> p a k f", a=a, k=2)
                v.tensor_scalar(wfv, cv, M_RNE, M_RNE, OP.add, OP.subtract)
                v.tensor_tensor(x0v, wfv, cv, OP.is_gt)
                v.tensor_tensor(x0v, wfv, x0v, OP.subtract)
                v.tensor_scalar(x0v, x0v, float(W - 2), None, OP.min)

            def emit_addr(a):
                """int32 quad indices -> I from X0[:, 0:a*64]."""
                x0v = X0[:, 0:a * 64].rearrange("p (a k f) -> p a k f",
                                                a=a, k=2)
                ifv = IF[:, 0:a * 32].rearrange("p (a one f) -> p a one f",
                                                a=a, one=1)
                iv = I[:, 0:a * 32].rearrange("p (a one f) -> p a one f",
                                              a=a, one=1)
                basev = (base_row.rearrange("p (one onee f) -> p one onee f",
                                            one=1, onee=1)
                         .to_broadcast([128, a, 1, 32]))
                v.scalar_tensor_tensor(ifv, x0v[:, :, 1:2, :],
                                       float(W), basev, OP.mult, OP.add)
                v.tensor_tensor(iv, ifv, x0v[:, :, 0:1, :], OP.add)

            def emit_idx(cv, a):
                emit_floor(cv, a, 0)
                emit_addr(a)

            def emit_weights(cv, a):
                """frac -> W4 m=1 (DVE); u/t = 1-frac -> W4 m=0 (ACT)."""
                x0v = X0[:, 0:a * 64].rearrange("p (a k f) -> p a k f",
                                                a=a, k=2)
                w4kv = W4[:, 0:a * 128].rearrange(
                    "p (a i k m) -> p a k i m", a=a, i=32, k=2, m=2)
                v.tensor_tensor(w4kv[:, :, :, :, 1:2], one1(cv), one1(x0v),
                                OP.subtract)
                nc.scalar.activation(w4kv[:, :, :, :, 0:1],
                                     w4kv[:, :, :, :, 1:2],
                                     AF.Copy, bias=1.0, scale=-1.0)

            def cand_coords_view():
                return (CAND[:].rearrange("p (a s) -> p a s", a=2)
                        [:, :, 0:64]
                        .rearrange("p a (k f) -> p a k f", k=2))

            def cand_score_view():
                return (CAND[:].rearrange("p (a s) -> p a s", a=2)
                        [:, :, 64:96])

            # ---- init: both host-precomputed gathers fire on the idx DMA
            emit_gather(0, 32, 0)      # initial coords
            emit_gather(32, 64, 128)   # first propagate (h|v)

            nc.vector.tensor_copy(BEST[:, 0:64], state[:, 0:64])
            nc.vector.tensor_copy(CAND[:, 0:64], state[:, 672:736])
            nc.vector.tensor_copy(CAND[:, 96:160], state[:, 736:800])

            emit_score(G[:, 0:128], state[:, 288:416],
                       BEST[:, 64:96].rearrange("p (a f) -> p a f", a=1), 1)
            emit_score(G[:, 128:384], state[:, 416:672], cand_score_view(), 2)

            def emit_accepts():
                v.tensor_tensor(UPD[:], b3(CAND[:, 64:96]), b3(BEST[:, 64:96]),
                                OP.is_gt)
                v.copy_predicated(BEST[:], UPD[:], CAND[:, 0:96])
                v.tensor_tensor(UPD[:], b3(CAND[:, 160:192]),
                                b3(BEST[:, 64:96]), OP.is_gt)
                v.copy_predicated(BEST[:], UPD[:], CAND[:, 96:192])

            emit_accepts()  # completes propagate(1,1)

            def propagate(dx, dy):
                # cand_v coords (vx|vy): partition shift on the idle PE via
                # an exact permutation matmul (1.0*x and 0.0*x products and
                # single-term sums are bit-exact even in decomposed fp32)
                pv = PERM[:, 0:128] if dy == 1 else PERM[:, 128:256]
                nc.tensor.matmul(PS[:], pv, BEST[:, 0:64],
                                 start=True, stop=True)

                # cand_h coords (hx|hy): col-roll by dx on Pool (parallel
                # with the PE v-roll)
                dv2 = CAND[:, 0:64].rearrange("p (c f) -> p c f", c=2)
                sv2 = BEST[:, 0:64].rearrange("p (c f) -> p c f", c=2)
                cp = nc.gpsimd.tensor_copy
                if dx == 1:
                    cp(dv2[:, :, 1:32], sv2[:, :, 0:31])
                    cp(dv2[64:128, :, 0:1], sv2[0:64, :, 31:32])
                    cp(dv2[0:64, :, 0:1], sv2[64:128, :, 31:32])
                else:
                    cp(dv2[:, :, 0:31], sv2[:, :, 1:32])
                    cp(dv2[0:64, :, 31:32], sv2[64:128, :, 0:1])
                    cp(dv2[64:128, :, 31:32], sv2[0:64, :, 0:1])
                if dx == 1:
                    v.tensor_scalar(CAND[:, 0:32], CAND[:, 0:32], 1.0,
                                    float(W - 1), OP.add, OP.min)
                else:
                    v.tensor_scalar(CAND[:, 0:32], CAND[:, 0:32], -1.0, 0.0,
                                    OP.add, OP.max)
                # h floor on DVE fills the PE-matmul + sem latency
                emit_floor(CAND[:, 0:64].rearrange("p (a k f) -> p a k f",
                                                   a=1, k=2), 1, 0)

                # v coords from PSUM, then v floor
                v.tensor_copy(CAND[:, 96:128], PS[:, 0:32])
                if dy == 1:
                    v.tensor_scalar(CAND[:, 128:160], PS[:, 32:64], 1.0,
                                    float(H - 1), OP.add, OP.min)
                else:
                    v.tensor_scalar(CAND[:, 128:160], PS[:, 32:64], -1.0,
                                    0.0, OP.add, OP.max)
                emit_floor(CAND[:, 96:160].rearrange("p (a k f) -> p a k f",
                                                     a=1, k=2), 1, 64)
                emit_addr(2)
                emit_gather_I(64, 128)
                emit_weights(cand_coords_view(), 2)
                emit_score(G[:, 128:384], W4[:, 0:256], cand_score_view(), 2)
                emit_accepts()

            def random_search(k):
                v.tensor_tensor(RC[:, 0:64], BEST[:, 0:64], noise_view(k),
                                OP.add)
                v.tensor_scalar(RC[:, 0:64], RC[:, 0:64], 0.0, float(W - 1),
                                OP.max, OP.min)
                cv = RC[:, 0:64].rearrange("p (a k f) -> p a k f", a=1, k=2)
                emit_idx(cv, 1)
                emit_gather_I(32, 0)
                emit_weights(cv, 1)
                emit_score(G[:, 0:128], W4[:, 0:128],
                           RC[:, 64:96].rearrange("p (a f) -> p a f", a=1), 1)
                v.tensor_tensor(UPD[:], b3(RC[:, 64:96]), b3(BEST[:, 64:96]),
                                OP.is_gt)
                v.copy_predicated(BEST[:], UPD[:], RC[:])

            random_search(0)
            propagate(-1, -1)
            random_search(1)
            propagate(-1, 1)
            random_search(2)
            propagate(1, -1)

            nc.sync.dma_start(out_xy.ap(), BEST[:, 0:64])

    nc.compile()
    return nc


def _get_program():
    if "nc" not in _CACHE:
        _CACHE["nc"] = _build_program()
    return _CACHE["nc"]


# ----------------------------------------------------------------------------
# Host-side helpers
# ----------------------------------------------------------------------------

def _to_layout(v):
    """[64(i), 64(j)] -> [128, 32]; partition = 64*(j//32)+i, free = j%32."""
    return np.ascontiguousarray(
        v.reshape(64, 2, 32).transpose(1, 0, 2).reshape(128, 32))


def _from_layout(a):
    """[128, 32] -> [64(i), 64(j)]."""
    return a.reshape(2, 64, 32).transpose(1, 0, 2).reshape(64, 64)


def _noise_arrays():
    """Mirror the reference's jax.random usage exactly, in-process."""
    import jax
    import jax.numpy as jnp

    key = jax.random.key(42)
    kf, kb = jax.random.split(key)
    out = []
    for kdir in (kf, kb):
        ks = jax.random.split(kdir, 3)
        out.append([np.asarray(R * jax.random.normal(k, (B, H, W, 2),
                                                     jnp.float32))
                    for k in ks])
    return out  # [dir][step] -> [B,H,W,2] float32


_PIX_BASE = None


def _pix_base():
    global _PIX_BASE
    if _PIX_BASE is None:
        _PIX_BASE = ((np.arange(64, dtype=np.int64)[:, None] * 64
                      + np.arange(64, dtype=np.int64)[None, :]) * PIX)
    return _PIX_BASE


def _quad_wi(x_plane, y_plane):
    """Clamped-floor weights + quad index for coords planes (fp32 mirror of
    the device arithmetic). Returns (W4 [128,128], idx [128,32] int32)."""
    one = np.float32(1.0)
    x0 = np.minimum(np.floor(x_plane), np.float32(W - 2)).astype(np.float32)
    y0 = np.minimum(np.floor(y_plane), np.float32(H - 2)).astype(np.float32)
    wx = x_plane - x0
    wy = y_plane - y0
    u = one - wx
    t = one - wy
    w4 = np.stack([_to_layout(u), _to_layout(wx),
                   _to_layout(t), _to_layout(wy)], axis=-1).reshape(128, 128)
    idx = (_pix_base()
           + y0.astype(np.int64) * 64 + x0.astype(np.int64))
    return np.ascontiguousarray(w4.astype(np.float32)), \
        _to_layout(idx).astype(np.int32)


def _clamp_np(x, lo, hi):
    return np.minimum(np.maximum(x, np.float32(lo)), np.float32(hi))


def _make_unit_inputs(x_plane, y_plane, noise_steps, b):
    """Build state [25,128,32] f32 and idx [128,96] i32 for one unit."""
    x_plane = x_plane.astype(np.float32)
    y_plane = y_plane.astype(np.float32)

    base_l = _to_layout(_pix_base().astype(np.float32))
    rows = [_to_layout(x_plane), _to_layout(y_plane), base_l]
    for step in range(3):
        nz = noise_steps[step][b]  # [H,W,2]
        rows.append(_to_layout(np.ascontiguousarray(nz[:, :, 0])))
        rows.append(_to_layout(np.ascontiguousarray(nz[:, :, 1])))

    w40, i0 = _quad_wi(x_plane, y_plane)
    rows.extend(w40.reshape(128, 4, 32).transpose(1, 0, 2))

    # first propagate (dx=1, dy=1) candidates, host-computed
    xh = _clamp_np(np.roll(x_plane, 1, axis=1) + np.float32(1.0), 0, W - 1)
    yh = _clamp_np(np.roll(y_plane, 1, axis=1), 0, H - 1)
    xv = _clamp_np(np.roll(x_plane, 1, axis=0), 0, W - 1)
    yv = _clamp_np(np.roll(y_plane, 1, axis=0) + np.float32(1.0), 0, H - 1)
    w4h, ih = _quad_wi(xh, yh)
    w4v, iv = _quad_wi(xv, yv)
    rows.extend(w4h.reshape(128, 4, 32).transpose(1, 0, 2))
    rows.extend(w4v.reshape(128, 4, 32).transpose(1, 0, 2))
    rows.extend([_to_layout(xh), _to_layout(yh),
                 _to_layout(xv), _to_layout(yv)])

    state = np.stack(rows).astype(np.float32)  # [25,128,32]
    state = np.ascontiguousarray(
        state.transpose(1, 0, 2).reshape(128, 25 * 32))
    idx = np.ascontiguousarray(
        np.concatenate([i0, ih, iv], axis=1)).astype(np.int32)
    return state, idx


def _perm_mats():
    """lhsT matrices for the PE v-roll: out[m] = in[src(m)], src within
    each 64-partition block with wraparound. [0]=dy+1, [1]=dy-1."""
    P = np.zeros((2, 128, 128), np.float32)
    for m in range(128):
        blk = (m // 64) * 64
        P[0, blk + ((m - blk - 1) % 64), m] = 1.0
        P[1, blk + ((m - blk + 1) % 64), m] = 1.0
    return np.ascontiguousarray(P.transpose(1, 0, 2).reshape(128, 256))


def _quad_corr(c):
    """c [4096, 64, 64] f32 -> quad [4096, 64, 64, 4]: per (y0, x0) the
    bilinear corner block [v00, v01, v10, v11] (edges padded by clamping;
    never addressed because x0, y0 <= 62)."""
    q = np.empty((PIX, H, W, 4), np.float32)
    q[..., 0] = c
    q[:, :, :-1, 1] = c[:, :, 1:]
    q[:, :, -1, 1] = c[:, :, -1]
    q[:, :-1, :, 2] = c[:, 1:, :]
    q[:, -1, :, 2] = c[:, -1, :]
    q[:, :-1, :-1, 3] = c[:, 1:, 1:]
    q[:, -1, :, 3] = q[:, -1, :, 1]
    q[:, :, -1, 3] = q[:, :, -1, 2]
    return q


def _bilinear_map_np(img, coords):
    """numpy mirror of reference._bilinear_map (fp32, same op order)."""
    Bn, Hn, Wn, C = img.shape
    out = np.empty_like(img)
    one = np.float32(1.0)
    for b in range(Bn):
        x = coords[b, :, :, 0].reshape(-1)
        y = coords[b, :, :, 1].reshape(-1)
        x0 = np.floor(x)
        y0 = np.floor(y)
        wx = (x - x0)[:, None]
        wy = (y - y0)[:, None]
        x0i = np.clip(x0.astype(np.int32), 0, Wn - 1)
        x1i = np.clip(x0i + 1, 0, Wn - 1)
        y0i = np.clip(y0.astype(np.int32), 0, Hn - 1)
        y1i = np.clip(y0i + 1, 0, Hn - 1)
        im = img[b]
        v00 = im[y0i, x0i]
        v01 = im[y0i, x1i]
        v10 = im[y1i, x0i]
        v11 = im[y1i, x1i]
        o = (v00 * (one - wx) * (one - wy) + v01 * wx * (one - wy)
             + v10 * (one - wx) * wy + v11 * wx * wy)
        out[b] = o.reshape(Hn, Wn, C)
    return out


def _run_device(in_maps, trace=False):
    from concourse import bass_utils

    nc = _get_program()
    res = bass_utils.run_bass_kernel_spmd(
        nc, in_maps, core_ids=list(range(N_CORES)), trace=trace)
    return res


def kernel(matching_f, matching_b, corr_map, _trace=False, _results_hook=None):
    matching_f = np.asarray(matching_f)
    matching_b = np.asarray(matching_b)
    corr_map = np.asarray(corr_map)

    noise = _noise_arrays()  # [dir][step][B,H,W,2]
    perm = _perm_mats()

    in_maps = []
    for b in range(B):  # forward units, cores 0..3
        cq = _quad_corr(np.ascontiguousarray(corr_map[b]).reshape(
            PIX, H, W))
        state, idx = _make_unit_inputs(matching_f[b, 0], matching_f[b, 1],
                                       noise[0], b)
        in_maps.append({"corr": cq.reshape(PIX * PIX, 4),
                        "state": state, "idx": idx, "perm": perm})
    for b in range(B):  # backward units, cores 4..7
        corr_t = np.ascontiguousarray(corr_map[b].transpose(2, 3, 0, 1))
        cq = _quad_corr(corr_t.reshape(PIX, H, W))
        state, idx = _make_unit_inputs(matching_b[b, 0], matching_b[b, 1],
                                       noise[1], b)
        in_maps.append({"corr": cq.reshape(PIX * PIX, 4),
                        "state": state, "idx": idx, "perm": perm})

    res = _run_device(in_maps, trace=_trace)
    if _results_hook is not None:
        _results_hook(res)

    res_f = np.empty((B, H, W, 2), np.float32)
    res_b = np.empty((B, H, W, 2), np.float32)
    for b in range(B):
        of = res.results[b]["out_xy"]
        ob = res.results[4 + b]["out_xy"]
        res_f[b, :, :, 0] = _from_layout(of[:, 0:32])
        res_f[b, :, :, 1] = _from_layout(of[:, 32:64])
        res_b[b, :, :, 0] = _from_layout(ob[:, 0:32])
        res_b[b, :, :, 1] = _from_layout(ob[:, 32:64])

    # forward-backward consistency (host; mirrors reference in fp32)
    counter = _bilinear_map_np(res_b, res_f)
    diff = np.max(np.abs(res_f - counter), axis=-1)
    invalid = (diff > EPS)[..., None]
    mf_t = matching_f.transpose(0, 2, 3, 1)  # [B,H,W,2]
    out = np.where(invalid, mf_t, res_f)
    return np.ascontiguousarray(out.transpose(0, 3, 1, 2)).astype(np.float32)


# revision 31
# speedup vs baseline: 1.1191x; 1.0147x over previous
"""PatchMatch-style MatchingPropagator on 8 Trainium2 NeuronCores.

Full inputs in, full outputs out. Sharding: 8 independent units =
(direction in {forward, backward}) x (batch 0..3), one NeuronCore each.
Core b runs forward for batch b; core 4+b runs backward for batch b using
the host-transposed correlation volume.

v2: the correlation volume is expanded on the host into a "quad" layout
corr_q[pixel, y0, x0] = [v00, v01, v10, v11] so one bilinear sample is a
single 16-byte indirect-DMA descriptor (half the SWDGE descriptor-
generation work, which is the dominant serial cost). The initial eval and
the first propagate depend only on the inputs, so their gather indices and
weight quads are precomputed on the host and shipped with the state; both
gathers issue as soon as the small index DMA lands. Scores are computed in
5 wide DVE ops via broadcast views of interleaved weight quads
[u, wx, t, wy], keeping the reference's exact fp32 multiply/add tree so
every argmax decision matches the reference bitwise. The gather address
chain runs on the Pool engine (which also issues the gather).

Pixel layout on chip: pixel (i, j) -> partition 64*(j//32) + i, free j%32.
"""

import numpy as np

B, H, W = 4, 64, 64
R = 3.0
EPS = np.float32(0.01)
N_CORES = 8
PIX = H * W  # 4096 pixels per unit; each owns a 64x64 correlation map
M_RNE = float(1 << 23)

_CACHE = {}


# ----------------------------------------------------------------------------
# Device program (SPMD: identical on all 8 cores; data differs per core)
# ----------------------------------------------------------------------------

def _build_program():
    import concourse.bass as bass
    import concourse.mybir as mybir
    import concourse.tile as tile
    from concourse import bacc

    F32 = mybir.dt.float32
    BF16 = mybir.dt.bfloat16
    I32 = mybir.dt.int32
    OP = mybir.AluOpType
    AF = mybir.ActivationFunctionType

    nc = bacc.Bacc(
        "TRN2",
        target_bir_lowering=False,
        debug=False,
        enable_asserts=False,
        num_devices=N_CORES,
    )

    # quad corr: row r = pixel*4096 + y0*64 + x0 -> [v00, v01, v10, v11]
    corr = nc.dram_tensor("corr", [PIX * PIX, 4], F32, kind="ExternalInput")
    # state rows (partition-major so the DMA is one contiguous run per
    # partition): 0 x, 1 y, 2 base, 3..8 noise (nx,ny)*3, 9..12 W40,
    # 13..20 W41, 21..22 cand_h xy, 23..24 cand_v xy
    state_in = nc.dram_tensor("state", [128, 25 * 32], F32,
                              kind="ExternalInput")
    idx_in = nc.dram_tensor("idx", [128, 96], I32, kind="ExternalInput")
    # partition-shift permutation matrices for the v-roll (dy=+1, dy=-1)
    perm_in = nc.dram_tensor("perm", [128, 256], F32,
                             kind="ExternalInput")
    out_xy = nc.dram_tensor("out_xy", [128, 64], F32,
                            kind="ExternalOutput")

    corr_ap = corr.ap()

    def b3(ap):  # [128,32] -> broadcast [128,3,32]
        return ap.rearrange("p (one f) -> p one f", one=1).to_broadcast(
            [128, 3, 32])

    with tile.TileContext(nc) as tc:
        with tc.tile_pool(name="main", bufs=1) as pool, \
                tc.tile_pool(name="ps", bufs=1,
                             space=bass.MemorySpace.PSUM) as ppool:
            IDX = pool.tile([128, 96], I32, name="IDX")
            state = pool.tile([128, 25 * 32], F32, name="state")
            PERM = pool.tile([128, 256], F32, name="PERM")
            PS = ppool.tile([128, 64], F32, name="PS")
            nc.sync.dma_start(IDX[:], idx_in.ap())
            nc.sync.dma_start(state[:], state_in.ap())
            nc.sync.dma_start(PERM[:], perm_in.ap())
            base_row = state[:, 64:96]

            def noise_view(k):
                o = 96 + 64 * k
                return state[:, o:o + 64]  # [nx|ny]

            BEST = pool.tile([128, 96], F32, name="BEST")    # [x|y|s]
            CAND = pool.tile([128, 192], F32, name="CAND")   # [hx hy hs|vx vy vs]
            RC = pool.tile([128, 96], F32, name="RC")        # [x y s]
            WF = pool.tile([128, 128], F32, name="WF")
            X0 = pool.tile([128, 128], F32, name="X0")
            X0I = pool.tile([128, 128], I32, name="X0I")
            W4 = pool.tile([128, 256], F32, name="W4")       # [u wx t wy]*
            IF = pool.tile([128, 64], F32, name="IF")
            I = pool.tile([128, 64], I32, name="I")
            G = pool.tile([128, 384], F32, name="G")         # single|dual
            At = pool.tile([128, 256], F32, name="At")
            Bt = pool.tile([128, 256], F32, name="Bt")
            D1 = pool.tile([128, 64], F32, name="D1")
            D2 = pool.tile([128, 64], F32, name="D2")
            UPD = pool.tile([128, 96], I32, name="UPD")

            v = nc.vector

            def emit_gather(icols_off, n_idx, gcols_off):
                nc.gpsimd.indirect_dma_start(
                    out=G[:, gcols_off:gcols_off + 4 * n_idx],
                    out_offset=None,
                    in_=corr_ap,
                    in_offset=bass.IndirectOffsetOnAxis(
                        ap=IDX[:, icols_off:icols_off + n_idx], axis=0),
                )

            def emit_gather_I(n_idx, gcols_off):
                nc.gpsimd.indirect_dma_start(
                    out=G[:, gcols_off:gcols_off + 4 * n_idx],
                    out_offset=None,
                    in_=corr_ap,
                    in_offset=bass.IndirectOffsetOnAxis(
                        ap=I[:, 0:n_idx], axis=0),
                )

            def one1(ap):  # append a trailing size-1 dim
                return ap.rearrange("... (f one) -> ... f one", one=1)

            def emit_score(g_flat, w_flat, out_v, a, eng=None, scr=0):
                """bilinear scores for `a` candidates: 5 ops, exact fp32
                tree ((v00*u)*t + (v01*wx)*t + (v10*u)*wy) + (v11*wx)*wy
                evaluated left-to-right like the reference. `scr` picks a
                scratch half so two engines can score concurrently."""
                e = eng or v
                g = g_flat.rearrange("p (a i r c) -> p a i r c",
                                     a=a, i=32, r=2, c=2)
                w4v = w_flat.rearrange("p (a i k m) -> p a i k m",
                                       a=a, i=32, k=2, m=2)
                uw = w4v[:, :, :, 0:1, :].to_broadcast([128, a, 32, 2, 2])
                twy = (w4v[:, :, :, 1:2, :]
                       .rearrange("p a i k m -> p a i m k")
                       .to_broadcast([128, a, 32, 2, 2]))
                ao, do = 128 * scr, 32 * scr
                atv = At[:, ao:ao + a * 128].rearrange(
                    "p (a i r c) -> p a i r c", a=a, i=32, r=2, c=2)
                btv = Bt[:, ao:ao + a * 128].rearrange(
                    "p (a i r c) -> p a i r c", a=a, i=32, r=2, c=2)
                e.tensor_tensor(atv, g, uw, OP.mult)
                e.tensor_tensor(btv, atv, twy, OP.mult)
                q = Bt[:, ao:ao + a * 128].rearrange(
                    "p (a i q) -> p a i q", a=a, i=32, q=4)
                # innermost-axis add-reduce accumulates left-to-right,
                # matching the reference's ((q0+q1)+q2)+q3 exactly
                e.tensor_reduce(out_v, q, mybir.AxisListType.X, OP.add)

            # largest fp32 below 63: min(c, this) then truncate == clamped floor
            F63M = float(np.nextafter(np.float32(W - 1), np.float32(0)))

            def emit_floor(cv, a, off):
                """cv: coords view [128, a, 2(k=x/y), 32] in [0, 63].
                Clamped floor -> X0[:, off:off+a*64] via the truncating
                f32->int32 dest conversion, then exact convert-back."""
                x0iv = X0I[:, off:off + a * 64].rearrange(
                    "p (a k f) -> p a k f", a=a, k=2)
                x0v = X0[:, off:off + a * 64].rearrange(
                    "p (a k f) -> p a k f", a=a, k=2)
                v.tensor_scalar(x0iv, cv, F63M, None, OP.min)
                v.tensor_copy(x0v, x0iv)

            def emit_addr(a):
                """int32 quad indices -> I from X0[:, 0:a*64]."""
                x0v = X0[:, 0:a * 64].rearrange("p (a k f) -> p a k f",
                                                a=a, k=2)
                ifv = IF[:, 0:a * 32].rearrange("p (a one f) -> p a one f",
                                                a=a, one=1)
                iv = I[:, 0:a * 32].rearrange("p (a one f) -> p a one f",
                                              a=a, one=1)
                basev = (base_row.rearrange("p (one onee f) -> p one onee f",
                                            one=1, onee=1)
                         .to_broadcast([128, a, 1, 32]))
                v.scalar_tensor_tensor(ifv, x0v[:, :, 1:2, :],
                                       float(W), basev, OP.mult, OP.add)
                v.tensor_tensor(iv, ifv, x0v[:, :, 0:1, :], OP.add)

            def emit_idx(cv, a):
                emit_floor(cv, a, 0)
                emit_addr(a)

            def emit_weights(cv, a):
                """frac -> W4 m=1 (DVE); u/t = 1-frac -> W4 m=0 (ACT)."""
                x0v = X0[:, 0:a * 64].rearrange("p (a k f) -> p a k f",
                                                a=a, k=2)
                w4kv = W4[:, 0:a * 128].rearrange(
                    "p (a i k m) -> p a k i m", a=a, i=32, k=2, m=2)
                v.tensor_tensor(w4kv[:, :, :, :, 1:2], one1(cv), one1(x0v),
                                OP.subtract)
                nc.scalar.activation(w4kv[:, :, :, :, 0:1],
                                     w4kv[:, :, :, :, 1:2],
                                     AF.Copy, bias=1.0, scale=-1.0)

            def cand_coords_view():
                return (CAND[:].rearrange("p (a s) -> p a s", a=2)
                        [:, :, 0:64]
                        .rearrange("p a (k f) -> p a k f", k=2))

            def cand_score_view():
                return (CAND[:].rearrange("p (a s) -> p a s", a=2)
                        [:, :, 64:96])

            # ---- init: both host-precomputed gathers fire on the idx DMA
            emit_gather(0, 32, 0)      # initial coords
            emit_gather(32, 64, 128)   # first propagate (h|v)

            nc.vector.tensor_copy(BEST[:, 0:64], state[:, 0:64])
            nc.vector.tensor_copy(CAND[:, 0:64], state[:, 672:736])
            nc.vector.tensor_copy(CAND[:, 96:160], state[:, 736:800])

            emit_score(G[:, 0:128], state[:, 288:416],
                       BEST[:, 64:96].rearrange("p (a f) -> p a f", a=1), 1)
            emit_score(G[:, 128:384], state[:, 416:672], cand_score_view(), 2)

            def emit_accepts():
                v.tensor_tensor(UPD[:], b3(CAND[:, 64:96]), b3(BEST[:, 64:96]),
                                OP.is_gt)
                v.copy_predicated(BEST[:], UPD[:], CAND[:, 0:96])
                v.tensor_tensor(UPD[:], b3(CAND[:, 160:192]),
                                b3(BEST[:, 64:96]), OP.is_gt)
                v.copy_predicated(BEST[:], UPD[:], CAND[:, 96:192])

            emit_accepts()  # completes propagate(1,1)

            def propagate(dx, dy):
                # cand_v coords (vx|vy): partition shift on the idle PE via
                # an exact permutation matmul (1.0*x and 0.0*x products and
                # single-term sums are bit-exact even in decomposed fp32)
                pv = PERM[:, 0:128] if dy == 1 else PERM[:, 128:256]
                nc.tensor.matmul(PS[:], pv, BEST[:, 0:64],
                                 start=True, stop=True)

                # cand_h coords (hx|hy): col-roll by dx on Pool (parallel
                # with the PE v-roll)
                dv2 = CAND[:, 0:64].rearrange("p (c f) -> p c f", c=2)
                sv2 = BEST[:, 0:64].rearrange("p (c f) -> p c f", c=2)
                cp = nc.gpsimd.tensor_copy
                if dx == 1:
                    cp(dv2[:, :, 1:32], sv2[:, :, 0:31])
                    cp(dv2[64:128, :, 0:1], sv2[0:64, :, 31:32])
                    cp(dv2[0:64, :, 0:1], sv2[64:128, :, 31:32])
                else:
                    cp(dv2[:, :, 0:31], sv2[:, :, 1:32])
                    cp(dv2[0:64, :, 31:32], sv2[64:128, :, 0:1])
                    cp(dv2[64:128, :, 31:32], sv2[0:64, :, 0:1])
                if dx == 1:
                    v.tensor_scalar(CAND[:, 0:32], CAND[:, 0:32], 1.0,
                                    float(W - 1), OP.add, OP.min)
                else:
                    v.tensor_scalar(CAND[:, 0:32], CAND[:, 0:32], -1.0, 0.0,
                                    OP.add, OP.max)
                # h floor on DVE fills the PE-matmul + sem latency
                emit_floor(CAND[:, 0:64].rearrange("p (a k f) -> p a k f",
                                                   a=1, k=2), 1, 0)

                # v coords from PSUM, then v floor
                v.tensor_copy(CAND[:, 96:128], PS[:, 0:32])
                if dy == 1:
                    v.tensor_scalar(CAND[:, 128:160], PS[:, 32:64], 1.0,
                                    float(H - 1), OP.add, OP.min)
                else:
                    v.tensor_scalar(CAND[:, 128:160], PS[:, 32:64], -1.0,
                                    0.0, OP.add, OP.max)
                emit_floor(CAND[:, 96:160].rearrange("p (a k f) -> p a k f",
                                                     a=1, k=2), 1, 64)
                emit_addr(2)
                emit_gather_I(64, 128)
                emit_weights(cand_coords_view(), 2)
                emit_score(G[:, 128:384], W4[:, 0:256], cand_score_view(), 2)
                emit_accepts()

            def random_search(k):
                v.tensor_tensor(RC[:, 0:64], BEST[:, 0:64], noise_view(k),
                                OP.add)
                v.tensor_scalar(RC[:, 0:64], RC[:, 0:64], 0.0, float(W - 1),
                                OP.max, OP.min)
                cv = RC[:, 0:64].rearrange("p (a k f) -> p a k f", a=1, k=2)
                emit_idx(cv, 1)
                emit_gather_I(32, 0)
                emit_weights(cv, 1)
                emit_score(G[:, 0:128], W4[:, 0:128],
                           RC[:, 64:96].rearrange("p (a f) -> p a f", a=1), 1)
                v.tensor_tensor(UPD[:], b3(RC[:, 64:96]), b3(BEST[:, 64:96]),
                                OP.is_gt)
                v.copy_predicated(BEST[:], UPD[:], RC[:])

            random_search(0)
            propagate(-1, -1)
            random_search(1)
            propagate(-1, 1)
            random_search(2)
            propagate(1, -1)

            nc.sync.dma_start(out_xy.ap(), BEST[:, 0:64])

    nc.compile()
    return nc


def _get_program():
    if "nc" not in _CACHE:
        _CACHE["nc"] = _build_program()
    return _CACHE["nc"]


# ----------------------------------------------------------------------------
# Host-side helpers
# ----------------------------------------------------------------------------

def _to_layout(v):
    """[64(i), 64(j)] -> [128, 32]; partition = 64*(j//32)+i, free = j%32."""
    return np.ascontiguousarray(
        v.reshape(64, 2, 32).transpose(1, 0, 2).reshape(128, 32))


def _from_layout(a):
    """[128, 32] -> [64(i), 64(j)]."""
    return a.reshape(2, 64, 32).transpose(1, 0, 2).reshape(64, 64)


def _noise_arrays():
    """Mirror the reference's jax.random usage exactly, in-process."""
    import jax
    import jax.numpy as jnp

    key = jax.random.key(42)
    kf, kb = jax.random.split(key)
    out = []
    for kdir in (kf, kb):
        ks = jax.random.split(kdir, 3)
        out.append([np.asarray(R * jax.random.normal(k, (B, H, W, 2),
                                                     jnp.float32))
                    for k in ks])
    return out  # [dir][step] -> [B,H,W,2] float32


_PIX_BASE = None


def _pix_base():
    global _PIX_BASE
    if _PIX_BASE is None:
        _PIX_BASE = ((np.arange(64, dtype=np.int64)[:, None] * 64
                      + np.arange(64, dtype=np.int64)[None, :]) * PIX)
    return _PIX_BASE


def _quad_wi(x_plane, y_plane):
    """Clamped-floor weights + quad index for coords planes (fp32 mirror of
    the device arithmetic). Returns (W4 [128,128], idx [128,32] int32)."""
    one = np.float32(1.0)
    x0 = np.minimum(np.floor(x_plane), np.float32(W - 2)).astype(np.float32)
    y0 = np.minimum(np.floor(y_plane), np.float32(H - 2)).astype(np.float32)
    wx = x_plane - x0
    wy = y_plane - y0
    u = one - wx
    t = one - wy
    w4 = np.stack([_to_layout(u), _to_layout(wx),
                   _to_layout(t), _to_layout(wy)], axis=-1).reshape(128, 128)
    idx = (_pix_base()
           + y0.astype(np.int64) * 64 + x0.astype(np.int64))
    return np.ascontiguousarray(w4.astype(np.float32)), \
        _to_layout(idx).astype(np.int32)


def _clamp_np(x, lo, hi):
    return np.minimum(np.maximum(x, np.float32(lo)), np.float32(hi))


def _make_unit_inputs(x_plane, y_plane, noise_steps, b):
    """Build state [25,128,32] f32 and idx [128,96] i32 for one unit."""
    x_plane = x_plane.astype(np.float32)
    y_plane = y_plane.astype(np.float32)

    base_l = _to_layout(_pix_base().astype(np.float32))
    rows = [_to_layout(x_plane), _to_layout(y_plane), base_l]
    for step in range(3):
        nz = noise_steps[step][b]  # [H,W,2]
        rows.append(_to_layout(np.ascontiguousarray(nz[:, :, 0])))
        rows.append(_to_layout(np.ascontiguousarray(nz[:, :, 1])))

    w40, i0 = _quad_wi(x_plane, y_plane)
    rows.extend(w40.reshape(128, 4, 32).transpose(1, 0, 2))

    # first propagate (dx=1, dy=1) candidates, host-computed
    xh = _clamp_np(np.roll(x_plane, 1, axis=1) + np.float32(1.0), 0, W - 1)
    yh = _clamp_np(np.roll(y_plane, 1, axis=1), 0, H - 1)
    xv = _clamp_np(np.roll(x_plane, 1, axis=0), 0, W - 1)
    yv = _clamp_np(np.roll(y_plane, 1, axis=0) + np.float32(1.0), 0, H - 1)
    w4h, ih = _quad_wi(xh, yh)
    w4v, iv = _quad_wi(xv, yv)
    rows.extend(w4h.reshape(128, 4, 32).transpose(1, 0, 2))
    rows.extend(w4v.reshape(128, 4, 32).transpose(1, 0, 2))
    rows.extend([_to_layout(xh), _to_layout(yh),
                 _to_layout(xv), _to_layout(yv)])

    state = np.stack(rows).astype(np.float32)  # [25,128,32]
    state = np.ascontiguousarray(
        state.transpose(1, 0, 2).reshape(128, 25 * 32))
    idx = np.ascontiguousarray(
        np.concatenate([i0, ih, iv], axis=1)).astype(np.int32)
    return state, idx


def _perm_mats():
    """lhsT matrices for the PE v-roll: out[m] = in[src(m)], src within
    each 64-partition block with wraparound. [0]=dy+1, [1]=dy-1."""
    P = np.zeros((2, 128, 128), np.float32)
    for m in range(128):
        blk = (m // 64) * 64
        P[0, blk + ((m - blk - 1) % 64), m] = 1.0
        P[1, blk + ((m - blk + 1) % 64), m] = 1.0
    return np.ascontiguousarray(P.transpose(1, 0, 2).reshape(128, 256))


def _quad_corr(c):
    """c [4096, 64, 64] f32 -> quad [4096, 64, 64, 4]: per (y0, x0) the
    bilinear corner block [v00, v01, v10, v11] (edges padded by clamping;
    never addressed because x0, y0 <= 62)."""
    q = np.empty((PIX, H, W, 4), np.float32)
    q[..., 0] = c
    q[:, :, :-1, 1] = c[:, :, 1:]
    q[:, :, -1, 1] = c[:, :, -1]
    q[:, :-1, :, 2] = c[:, 1:, :]
    q[:, -1, :, 2] = c[:, -1, :]
    q[:, :-1, :-1, 3] = c[:, 1:, 1:]
    q[:, -1, :, 3] = q[:, -1, :, 1]
    q[:, :, -1, 3] = q[:, :, -1, 2]
    return q


def _bilinear_map_np(img, coords):
    """numpy mirror of reference._bilinear_map (fp32, same op order)."""
    Bn, Hn, Wn, C = img.shape
    out = np.empty_like(img)
    one = np.float32(1.0)
    for b in range(Bn):
        x = coords[b, :, :, 0].reshape(-1)
        y = coords[b, :, :, 1].reshape(-1)
        x0 = np.floor(x)
        y0 = np.floor(y)
        wx = (x - x0)[:, None]
        wy = (y - y0)[:, None]
        x0i = np.clip(x0.astype(np.int32), 0, Wn - 1)
        x1i = np.clip(x0i + 1, 0, Wn - 1)
        y0i = np.clip(y0.astype(np.int32), 0, Hn - 1)
        y1i = np.clip(y0i + 1, 0, Hn - 1)
        im = img[b]
        v00 = im[y0i, x0i]
        v01 = im[y0i, x1i]
        v10 = im[y1i, x0i]
        v11 = im[y1i, x1i]
        o = (v00 * (one - wx) * (one - wy) + v01 * wx * (one - wy)
             + v10 * (one - wx) * wy + v11 * wx * wy)
        out[b] = o.reshape(Hn, Wn, C)
    return out


def _run_device(in_maps, trace=False):
    from concourse import bass_utils

    nc = _get_program()
    res = bass_utils.run_bass_kernel_spmd(
        nc, in_maps, core_ids=list(range(N_CORES)), trace=trace)
    return res


def kernel(matching_f, matching_b, corr_map, _trace=False, _results_hook=None):
    matching_f = np.asarray(matching_f)
    matching_b = np.asarray(matching_b)
    corr_map = np.asarray(corr_map)

    noise = _noise_arrays()  # [dir][step][B,H,W,2]
    perm = _perm_mats()

    in_maps = []
    for b in range(B):  # forward units, cores 0..3
        cq = _quad_corr(np.ascontiguousarray(corr_map[b]).reshape(
            PIX, H, W))
        state, idx = _make_unit_inputs(matching_f[b, 0], matching_f[b, 1],
                                       noise[0], b)
        in_maps.append({"corr": cq.reshape(PIX * PIX, 4),
                        "state": state, "idx": idx, "perm": perm})
    for b in range(B):  # backward units, cores 4..7
        corr_t = np.ascontiguousarray(corr_map[b].transpose(2, 3, 0, 1))
        cq = _quad_corr(corr_t.reshape(PIX, H, W))
        state, idx = _make_unit_inputs(matching_b[b, 0], matching_b[b, 1],
                                       noise[1], b)
        in_maps.append({"corr": cq.reshape(PIX * PIX, 4),
                        "state": state, "idx": idx, "perm": perm})

    res = _run_device(in_maps, trace=_trace)
    if _results_hook is not None:
        _results_hook(res)

    res_f = np.empty((B, H, W, 2), np.float32)
    res_b = np.empty((B, H, W, 2), np.float32)
    for b in range(B):
        of = res.results[b]["out_xy"]
        ob = res.results[4 + b]["out_xy"]
        res_f[b, :, :, 0] = _from_layout(of[:, 0:32])
        res_f[b, :, :, 1] = _from_layout(of[:, 32:64])
        res_b[b, :, :, 0] = _from_layout(ob[:, 0:32])
        res_b[b, :, :, 1] = _from_layout(ob[:, 32:64])

    # forward-backward consistency (host; mirrors reference in fp32)
    counter = _bilinear_map_np(res_b, res_f)
    diff = np.max(np.abs(res_f - counter), axis=-1)
    invalid = (diff > EPS)[..., None]
    mf_t = matching_f.transpose(0, 2, 3, 1)  # [B,H,W,2]
    out = np.where(invalid, mf_t, res_f)
    return np.ascontiguousarray(out.transpose(0, 3, 1, 2)).astype(np.float32)
